# revision 1
# baseline (speedup 1.0000x reference)
"""Trainium2 Bass kernel: transformer block (LN->attn->LN->MLP, pre-norm residual).

Sharding: 8 cores, zero collectives. Core c handles batch b=c//2, query-token
half h=c%2 (1024 q-tokens). Each core computes LN1 + K/V over its batch's full
2048 tokens (duplicated within the pair), Q/attention/proj/MLP only for its
1024 tokens. Host rolls tokens so the q-half is always tokens 0..1023 (softmax
is permutation-invariant over keys), keeping one SPMD program for all cores.

Numerics: bf16 matmul operands, fp32 PSUM accumulation, fp32 residual stream.
LN gain/bias and all linear biases are folded on the host:
  - LN gain g folds into the following weight (W*g), LN bias b into a bias
    (W@b + lin_b).
  - k-bias drops entirely (softmax shift invariance); q-bias applied at the
    q PSUM evict; v-bias commutes through softmax (rows sum to 1) and folds,
    together with proj_b, into the attention residual added on the host side
    (x_res += proj_b + proj_w @ bv_eff).
  - fc1 bias applied at the gelu evict; fc2 bias added via a K=1 ones matmul.

SBUF is managed with LIFO phase-scoped tile pools; the fp32 residual y1
stays resident in SBUF across both halves.
"""

import numpy as np
import ml_dtypes
from contextlib import ExitStack

import concourse.bass as bass
import concourse.tile as tile
from concourse import bacc, mybir
from concourse.bass_utils import run_bass_kernel_spmd

F32 = mybir.dt.float32
BF16 = mybir.dt.bfloat16
AF = mybir.ActivationFunctionType
ALU = mybir.AluOpType

DIM = 768
NH = 12
HD = 64
HID = 3072
B = 4
T = 2048
TQ = 1024
NCORES = 8
EPS = 1e-6

KC = DIM // 128     # 6  contraction chunks over model dim
HC = HID // 128     # 24 contraction chunks over hidden dim
NTB = T // 128      # 16 token blocks (full batch)
NQB = TQ // 128     # 8  q-token blocks
HP = NH // 2        # 6  head pairs


DEBUG_DUMPS = False


def _emit(nc, tc, ctx, d):
    """Emit the whole per-core program into the TileContext."""
    P = 128

    def dump(name, ap):
        if DEBUG_DUMPS:
            nc.sync.dma_start(d[name], ap)

    # ---- whole-kernel pools ----
    outp = ctx.enter_context(tc.tile_pool(name="outer", bufs=1))
    statp = ctx.enter_context(tc.tile_pool(name="stats", bufs=8))
    yop = ctx.enter_context(tc.tile_pool(name="yout", bufs=2))

    ones1 = outp.tile([1, P], BF16, tag="ones1")
    ident = outp.tile([P, P], BF16, tag="ident")
    epst = outp.tile([P, 1], F32, tag="epst")
    bfc2 = outp.tile([1, DIM], BF16, tag="bfc2")
    y1 = outp.tile([P, NQB, DIM], F32, tag="y1")   # fp32 residual stream

    nc.gpsimd.memset(ones1[:, :], 1.0)
    nc.gpsimd.memset(epst[:, :], EPS)
    sqwarm = outp.tile([1, 1], F32, tag="sqwarm")
    nc.scalar.activation(sqwarm[:, :], epst[0:1, :], AF.Sqrt)
    nc.sync.dma_start(bfc2[:, :], d["bfc2"])
    nc.sync.dma_start(ident[:, :], d["ident"])

    def pe_transpose_tile(xh_tile, dstT, tb, pspool, pstag):
        for kc in range(KC):
            pt = pspool.tile([P, P], BF16, tag=pstag, name=f"pt{tb}_{kc}")
            nc.tensor.transpose(
                pt[:], xh_tile[:, kc * 128:(kc + 1) * 128], ident[:, :])
            nc.scalar.copy(dstT[:, kc, tb * 128:(tb + 1) * 128], pt[:])

    def layernorm_tile(src_ap, xh_tile):
        st = statp.tile([P, 2, 6], F32, tag="st")
        nc.vector.bn_stats(st[:, 0, :], src_ap[:, 0:384])
        nc.vector.bn_stats(st[:, 1, :], src_ap[:, 384:768])
        ag = statp.tile([P, 2], F32, tag="ag")
        nc.vector.bn_aggr(ag[:], st[:])
        sd = statp.tile([P, 1], F32, tag="sd")
        nc.scalar.activation(sd[:], ag[:, 1:2], AF.Sqrt, bias=epst[:, :])
        rs = statp.tile([P, 1], F32, tag="rs")
        nc.vector.reciprocal(rs[:], sd[:])
        nc.vector.tensor_scalar(
            xh_tile[:], src_ap, ag[:, 0:1], rs[:], ALU.subtract, ALU.mult
        )

    # ================= attention half =================
    with tc.tile_pool(name="atn_keep", bufs=1) as keepp:
        aT = keepp.tile([P, KC, TQ], BF16, tag="aT")   # normalized attn out^T

        with tc.tile_pool(name="atn", bufs=1) as atp, \
             tc.tile_pool(name="exp", bufs=4) as expp, \
             tc.tile_pool(name="rec", bufs=1) as recp, \
             tc.tile_pool(name="dnp", bufs=3) as dnp, \
             tc.tile_pool(name="recb", bufs=3) as recbp, \
             tc.tile_pool(name="tmpodd", bufs=2) as tmpp, \
             tc.tile_pool(name="ps_sc", bufs=2, space="PSUM") as ps_sc, \
             tc.tile_pool(name="ps_av", bufs=4, space="PSUM") as ps_av:

            ones1f = atp.tile([1, 64], BF16, tag="ones1f")
            nc.gpsimd.memset(ones1f[:, :], 1.0)
            qT = atp.tile([P, KC, TQ], BF16, tag="qT")
            kT = atp.tile([P, KC, T], BF16, tag="kT")
            vp = atp.tile([P, NTB, NH, 65], BF16, tag="vp")
            nc.gpsimd.memset(vp[:, :, :, 64:65], 1.0)

            # ---- phase A: LN1 + QKV projections ----
            with tc.tile_pool(name="qkv", bufs=1) as qkvp, \
                 tc.tile_pool(name="xtok", bufs=2) as xtokp, \
                 tc.tile_pool(name="xhat", bufs=2) as xhp:
                ps_qkv = ps_sc  # borrow the attention score psum slots

                wq = qkvp.tile([P, KC, DIM], BF16, tag="wq")
                wk = qkvp.tile([P, KC, DIM], BF16, tag="wk")
                wv = qkvp.tile([P, KC, DIM], BF16, tag="wv")
                bq = qkvp.tile([P, KC, 1], F32, tag="bq")
                xT = qkvp.tile([P, KC, T], BF16, tag="xT")

                # LN1 over all T tokens -> xT (feature-major, bf16).
                # x loads + weight loads on the ACT hwdge queue, transposes
                # on the SP queue; first 4 token blocks unblock the first
                # k/q matmul chunk, then weights, then the rest.
                xgs = []
                for g in range(NTB // 2):
                    xg = xtokp.tile([P, 2, DIM], F32, tag="xt", name=f"xg{g}",
                                    bufs=3)
                    nc.sync.dma_start(
                        xg[:], d["x_tok"][2 * g:2 * g + 2]
                        .rearrange("t p f -> p t f"))
                    xgs.append(xg)
                    if g == 1:
                        nc.sync.dma_start(
                            wk[:], d["wk"].rearrange("k p f -> p k f"))
                        nc.sync.dma_start(
                            wq[:], d["wq"].rearrange("k p f -> p k f"))
                nc.sync.dma_start(wv[:], d["wv"].rearrange("k p f -> p k f"))
                nc.sync.dma_start(bq[:, :, :],
                                  d["bq"].rearrange("k p o -> p k o"))
                for tb in range(NTB):
                    xh = xhp.tile([P, DIM], BF16, tag="xh")
                    layernorm_tile(xgs[tb // 2][:, tb % 2, :], xh)
                    pe_transpose_tile(xh, xT, tb, ps_av, "av")

                for nc2 in range(T // 512):  # 512-token chunks
                    tsl = slice(nc2 * 512, (nc2 + 1) * 512)
                    # k^T
                    for mb in range(KC):
                        ps = ps_qkv.tile([P, 512], F32, tag="sc")
                        for kc in range(KC):
                            nc.tensor.matmul(
                                ps[:], wk[:, kc, mb * 128 : (mb + 1) * 128],
                                xT[:, kc, tsl],
                                start=(kc == 0), stop=(kc == KC - 1),
                            )
                        nc.vector.tensor_copy(kT[:, mb, tsl], ps[:])
                    # q^T (first TQ tokens only), scaled 1/8 with folded bias
                    if nc2 < TQ // 512:
                        for mb in range(KC):
                            ps = ps_qkv.tile([P, 512], F32, tag="sc")
                            for kc in range(KC):
                                nc.tensor.matmul(
                                    ps[:], wq[:, kc, mb * 128 : (mb + 1) * 128],
                                    xT[:, kc, tsl],
                                    start=(kc == 0), stop=(kc == KC - 1),
                                )
                            nc.vector.tensor_scalar(
                                qT[:, mb, tsl], ps[:], float(HD) ** -0.5,
                                bq[:, mb, :], ALU.mult, ALU.add,
                            )
                    # v (token-major) into the per-head 65-col layout
                    for tb in range(nc2 * 4, nc2 * 4 + 4):
                        for c0, c1 in ((0, 512), (512, 768)):
                            ps = ps_qkv.tile([P, c1 - c0], F32, tag="sc")
                            for kc in range(KC):
                                nc.tensor.matmul(
                                    ps[:], xT[:, kc, tb * 128 : (tb + 1) * 128],
                                    wv[:, kc, c0:c1],
                                    start=(kc == 0), stop=(kc == KC - 1),
                                )
                            h0, h1 = c0 // HD, c1 // HD
                            pr = ps[:].rearrange("p (h c) -> p h c", c=HD)
                            nc.vector.tensor_copy(vp[:, tb, h0:h1, 0:64],
                                                   pr[:, :, :])

                dump("d_xT", xT[:])

            dump("d_qT", qT[:])
            dump("d_kT", kT[:])
            dump("d_vp", vp[:])

            # ---- phase B: attention, (pair, q-chunk) outer / k-block inner.
            # The normalize chain for group g is emitted after group g+1's
            # accumulation loop so its PE/DVE/ACT legs overlap instead of
            # stalling the PE stream (HAM stays warm).
            def emit_normalize(p, qc, avA, avB):
                qsl = slice(qc * 512, (qc + 1) * 512)
                rec = recp.tile([65, 2, 512], F32, tag="rec",
                                name=f"rec{p}_{qc}")
                nc.vector.tensor_copy(rec[64:65, 0, :], avA[64:65, :])
                nc.vector.tensor_copy(rec[64:65, 1, :], avB[64:65, :])
                dns = dnp.tile([128, 8], F32, tag="dns", name=f"dns{p}_{qc}")
                nc.sync.dma_start(dns[:, :], rec[64:65, :, :])
                dnr = dnp.tile([128, 8], BF16, tag="dnr", name=f"dnr{p}_{qc}")
                with nc.allow_low_precision(reason="softmax denom recip"):
                    nc.vector.reciprocal(dnr[:, :], dns[:, :])
                rrow = dnp.tile([1, 2, 512], BF16, tag="rrow",
                                name=f"rrow{p}_{qc}")
                nc.sync.dma_start(rrow[0:1, :, :], dnr[:, :])
                for par, av in ((0, avA), (1, avB)):
                    psb = ps_sc.tile([64, 512], F32, tag="sc",
                                     name=f"psb{p}_{qc}_{par}")
                    nc.tensor.matmul(psb[:, :], ones1f[0:1, :],
                                     rrow[0:1, par, :])
                    recb = recbp.tile([64, 512], F32, tag="recb",
                                      name=f"recb{p}_{qc}_{par}")
                    nc.scalar.copy(recb[0:64, :], psb[0:64, :])
                    if par == 0:
                        nc.vector.tensor_tensor(
                            aT[0:64, p, qsl], av[0:64, :],
                            recb[0:64, :], op=ALU.mult,
                        )
                    else:
                        tmp = tmpp.tile([64, 512], BF16, tag="tmpodd")
                        nc.vector.tensor_tensor(
                            tmp[0:64, :], av[0:64, :],
                            recb[0:64, :], op=ALU.mult,
                        )
                        nc.sync.dma_start(aT[64:128, p, qsl], tmp[0:64, :])

            pending = None
            for p in range(HP):
                hA, hB = 2 * p, 2 * p + 1
                for qc in range(2):
                    qsl = slice(qc * 512, (qc + 1) * 512)
                    avA = ps_av.tile([65, 512], F32, tag="av",
                                     name=f"avA{p}_{qc}")
                    avB = ps_av.tile([65, 512], F32, tag="av",
                                     name=f"avB{p}_{qc}")
                    for kb in range(NTB):
                        ksl = slice(kb * 128, (kb + 1) * 128)
                        ex = expp.tile([P, 2, 512], BF16, tag="ex")
                        psS = ps_sc.tile([P, 2, 512], F32, tag="sc",
                                         name=f"scS{p}_{qc}_{kb}")
                        nc.tensor.matmul(
                            psS[:, 0, :], kT[0:64, p, ksl], qT[0:64, p, qsl],
                            tile_position=(0, 0),
                        )
                        nc.tensor.matmul(
                            psS[:, 1, :], kT[64:128, p, ksl], qT[64:128, p, qsl],
                            tile_position=(64, 0),
                        )
                        nc.scalar.activation(ex[:, :, :], psS[:, :, :], AF.Exp)
                        nc.tensor.matmul(
                            avA[:], vp[:, kb, hA, :], ex[:, 0, :],
                            start=(kb == 0), stop=(kb == NTB - 1),
                        )
                        nc.tensor.matmul(
                            avB[:], vp[:, kb, hB, :], ex[:, 1, :],
                            start=(kb == 0), stop=(kb == NTB - 1),
                        )
                    if pending is not None:
                        emit_normalize(*pending)
                    pending = (p, qc, avA, avB)
            emit_normalize(*pending)

        dump("d_aT", aT[:])

        # ---- phase C: proj + residual -> y1 (token-major fp32, to DRAM) ----
        # keep y1 in fp32 SBUF across proj -> LN2 -> fc2 (no DRAM roundtrip)
        with tc.tile_pool(name="proj", bufs=1) as prp, \
             tc.tile_pool(name="ps_mm", bufs=4, space="PSUM") as ps_mm:
            wproj = prp.tile([P, KC, DIM], BF16, tag="wproj")
            nc.scalar.dma_start(wproj[:],
                                d["wproj"].rearrange("k p f -> p k f"))
            xresb = prp.tile([P, NQB, DIM], F32, tag="xresb")
            nc.scalar.dma_start(xresb[:],
                                d["x_res"].rearrange("t p f -> p t f"))
            for mb in range(NQB):
                msl = slice(mb * 128, (mb + 1) * 128)
                ps0 = ps_mm.tile([P, 512], F32, tag="psmm")
                ps1 = ps_mm.tile([P, 256], F32, tag="psmm")
                for kc in range(KC):
                    nc.tensor.matmul(
                        ps0[:], aT[:, kc, msl], wproj[:, kc, 0:512],
                        start=(kc == 0), stop=(kc == KC - 1),
                    )
                    nc.tensor.matmul(
                        ps1[:], aT[:, kc, msl], wproj[:, kc, 512:768],
                        start=(kc == 0), stop=(kc == KC - 1),
                    )
                nc.vector.tensor_tensor(y1[:, mb, 0:512], ps0[:],
                                        xresb[:, mb, 0:512], op=ALU.add)
                nc.vector.tensor_tensor(y1[:, mb, 512:768], ps1[:],
                                        xresb[:, mb, 512:768], op=ALU.add)
                if DEBUG_DUMPS:
                    nc.sync.dma_start(d["d_y1"][mb], y1[:, mb, :])

    # ================= MLP half =================
    with tc.tile_pool(name="mlp_keep", bufs=1) as mkp:
        hT = mkp.tile([P, HC, TQ], BF16, tag="hT")
        wfc2 = mkp.tile([P, HC, DIM], BF16, tag="wfc2")

        # ---- phase D: LN2 + fc1 + gelu -> hT ----
        with tc.tile_pool(name="fc1", bufs=1) as f1p, \
             tc.tile_pool(name="xhat2", bufs=2) as xh2p, \
             tc.tile_pool(name="ps_fc1", bufs=4, space="PSUM") as ps_fc1:
            wfc1 = f1p.tile([P, KC, HID], BF16, tag="wfc1")
            bfc1 = f1p.tile([P, HC, 1], F32, tag="bfc1")
            x2T = f1p.tile([P, KC, TQ], BF16, tag="x2T")
            nc.scalar.dma_start(wfc1[:],
                                d["wfc1"].rearrange("k p f -> p k f"))
            nc.scalar.dma_start(bfc1[:, :, :],
                                d["bfc1"].rearrange("k p o -> p k o"))

            for mb in range(NQB):
                xh = xh2p.tile([P, DIM], BF16, tag="xh2")
                layernorm_tile(y1[:, mb, :], xh)
                pe_transpose_tile(xh, x2T, mb, ps_fc1, "psfc1")

            dump("d_x2T", x2T[:])
            for hb in range(HC):
                for qc in range(2):
                    qsl = slice(qc * 512, (qc + 1) * 512)
                    ps = ps_fc1.tile([P, 512], F32, tag="psfc1")
                    for kc in range(KC):
                        nc.tensor.matmul(
                            ps[:], wfc1[:, kc, hb * 128 : (hb + 1) * 128],
                            x2T[:, kc, qsl],
                            start=(kc == 0), stop=(kc == KC - 1),
                        )
                    nc.scalar.activation(hT[:, hb, qsl], ps[:], AF.Gelu,
                                         bias=bfc1[:, hb, :])

        dump("d_hT", hT[:])

        nc.scalar.dma_start(wfc2[:], d["wfc2"].rearrange("k p f -> p k f"))

        # ---- phase E: fc2 + bias + residual -> y_out ----
        with tc.tile_pool(name="ps_fc2", bufs=4, space="PSUM") as ps_fc2:
            for mb in range(NQB):
                msl = slice(mb * 128, (mb + 1) * 128)
                ps0 = ps_fc2.tile([P, 512], F32, tag="psfc2")
                ps1 = ps_fc2.tile([P, 256], F32, tag="psfc2")
                for kc in range(HC):
                    nc.tensor.matmul(
                        ps0[:], hT[:, kc, msl], wfc2[:, kc, 0:512],
                        start=(kc == 0), stop=False,
                    )
                    nc.tensor.matmul(
                        ps1[:], hT[:, kc, msl], wfc2[:, kc, 512:768],
                        start=(kc == 0), stop=False,
                    )
                nc.tensor.matmul(ps0[:], ones1[0:1, :], bfc2[0:1, 0:512],
                                 start=False, stop=True)
                nc.tensor.matmul(ps1[:], ones1[0:1, :], bfc2[0:1, 512:768],
                                 start=False, stop=True)
                yo = yop.tile([P, DIM], F32, tag="yo")
                nc.vector.tensor_tensor(yo[:, 0:512], ps0[:],
                                        y1[:, mb, 0:512], op=ALU.add)
                nc.vector.tensor_tensor(yo[:, 512:768], ps1[:],
                                        y1[:, mb, 512:768], op=ALU.add)
                nc.sync.dma_start(d["y_out"][mb], yo[:])


_PROGRAM = None


def build_program():
    global _PROGRAM
    if _PROGRAM is not None:
        return _PROGRAM
    nc = bacc.Bacc("TRN2", debug=False, target_bir_lowering=False,
                   num_devices=NCORES)
    d = {}

    def din(name, shape, dt):
        d[name] = nc.dram_tensor(name, shape, dt, kind="ExternalInput").ap()

    din("x_tok", [NTB, 128, DIM], F32)
    din("x_res", [NQB, 128, DIM], F32)
    din("wq", [KC, 128, DIM], BF16)
    din("wk", [KC, 128, DIM], BF16)
    din("wv", [KC, 128, DIM], BF16)
    din("bq", [KC, 128, 1], F32)
    din("wproj", [KC, 128, DIM], BF16)
    din("wfc1", [KC, 128, HID], BF16)
    din("bfc1", [HC, 128, 1], F32)
    din("wfc2", [HC, 128, DIM], BF16)
    din("bfc2", [1, DIM], BF16)
    din("ident", [128, 128], BF16)
    d["y_out"] = nc.dram_tensor("y_out", [NQB, 128, DIM], F32,
                                kind="ExternalOutput").ap()
    if DEBUG_DUMPS:
        def dout(name, shape, dt):
            d[name] = nc.dram_tensor(name, shape, dt,
                                     kind="ExternalOutput").ap()
        dout("d_xT", [128, KC, T], BF16)
        dout("d_qT", [128, KC, TQ], BF16)
        dout("d_kT", [128, KC, T], BF16)
        dout("d_vp", [128, NTB, NH, 65], BF16)
        dout("d_aT", [128, KC, TQ], BF16)
        dout("d_ex0", [128, 2, TQ], BF16)
        dout("d_av", [2, 65, TQ], F32)
        dout("d_rec", [2, 1, TQ], F32)
        dout("d_recb", [2, 64, TQ], F32)
        dout("d_y1", [NQB, 128, DIM], F32)
        dout("d_x2T", [128, KC, TQ], BF16)
        dout("d_hT", [128, HC, TQ], BF16)

    with tile.TileContext(nc) as tc:
        with ExitStack() as ctx:
            _emit(nc, tc, ctx, d)
    nc.compile()
    _PROGRAM = nc
    return nc


def _prep_in_maps(inputs):
    f32 = lambda a: np.ascontiguousarray(np.asarray(a, dtype=np.float32))
    bf = lambda a: np.ascontiguousarray(
        np.asarray(a, dtype=np.float32).astype(ml_dtypes.bfloat16))

    x = f32(inputs["x"])
    g1, b1 = f32(inputs["ln1_g"]), f32(inputs["ln1_b"])
    qkv_w, qkv_b = f32(inputs["qkv_w"]), f32(inputs["qkv_b"])
    proj_w, proj_b = f32(inputs["proj_w"]), f32(inputs["proj_b"])
    g2, b2 = f32(inputs["ln2_g"]), f32(inputs["ln2_b"])
    fc1_w, fc1_b = f32(inputs["fc1_w"]), f32(inputs["fc1_b"])
    fc2_w, fc2_b = f32(inputs["fc2_w"]), f32(inputs["fc2_b"])

    Wq, Wk, Wv = qkv_w[:DIM], qkv_w[DIM:2 * DIM], qkv_w[2 * DIM:]
    bq_eff = (qkv_b[:DIM] + Wq @ b1) * (float(HD) ** -0.5)
    bv_eff = qkv_b[2 * DIM:] + Wv @ b1
    xres_const = proj_b + proj_w @ bv_eff

    shared = {
        "ident": bf(np.eye(128, dtype=np.float32)),
        "wq": bf((Wq * g1).T.reshape(KC, 128, DIM)),
        "wk": bf((Wk * g1).T.reshape(KC, 128, DIM)),
        "wv": bf((Wv * g1).T.reshape(KC, 128, DIM)),
        "bq": f32(bq_eff.reshape(KC, 128, 1)),
        "wproj": bf(proj_w.T.reshape(KC, 128, DIM)),
        "wfc1": bf((fc1_w * g2).T.reshape(KC, 128, HID)),
        "bfc1": f32((fc1_b + fc1_w @ b2).reshape(HC, 128, 1)),
        "wfc2": bf(fc2_w.T.reshape(HC, 128, DIM)),
        "bfc2": bf(fc2_b.reshape(1, DIM)),
    }
    in_maps = []
    for c in range(NCORES):
        b, h = divmod(c, 2)
        xr = np.roll(x[b], -h * TQ, axis=0)
        m = dict(shared)
        m["x_tok"] = np.ascontiguousarray(xr.reshape(NTB, 128, DIM))
        m["x_res"] = np.ascontiguousarray(
            (xr[:TQ] + xres_const).reshape(NQB, 128, DIM))
        in_maps.append(m)
    return in_maps


def run(inputs, trace=False, **kwargs):
    nc = build_program()
    in_maps = _prep_in_maps(inputs)
    res = run_bass_kernel_spmd(nc, in_maps, core_ids=list(range(NCORES)),
                               trace=trace, **kwargs)
    out = np.empty((B, T, DIM), np.float32)
    for c in range(NCORES):
        b, h = divmod(c, 2)
        out[b, h * TQ:(h + 1) * TQ] = (
            res.results[c]["y_out"].reshape(TQ, DIM).astype(np.float32))
    return out, res


def kernel(**inputs) -> np.ndarray:
    out, _ = run(inputs, trace=False)
    return out

